# revision 55
# baseline (speedup 1.0000x reference)
# MoE (top-2 routed experts + shared expert SwiGLU) on 8 TRN2 NeuronCores.
#
# Sharding: expert-parallel. Core e owns expert e's FFN weights and processes
# the tokens routed to expert e (padded to a fixed capacity); the shared
# expert runs data-parallel (each core takes T/8 tokens with replicated
# shared weights). Routing (sigmoid gate -> top-2 -> stable sort by expert)
# is part of the host-side sharding step: it decides which token goes to
# which core, exactly mirroring the reference's jax ops so expert selection
# is bit-identical. All FFN GEMMs (99.9% of FLOPs) run on device in bf16
# with fp32 PSUM accumulation, matching the reference's bf16 expert compute.
#
# Device layout: tokens live on the matmul free dim (everything pre-transposed
# host-side), weights stream as [128, free] k-tiles used as lhsT slices.
import os
import sys
import tempfile

import numpy as np
import ml_dtypes

for _p in ("/opt/trn_rl_repo", "/root/.axon_site/_ro/trn_rl_repo"):
    if os.path.isdir(_p) and _p not in sys.path:
        sys.path.append(_p)

BF16 = ml_dtypes.bfloat16

P = 128
D = 2048          # model dim
H = 1024          # ffn hidden dim
T = 2048          # batch*seq tokens
E = 8             # experts == cores
TOPK = 2
C = 556           # per-expert token capacity (max observed count 554; overflow has a numpy fallback)
S = T // 8        # shared-expert tokens per core
KD = D // P       # 16 k-tiles over D
KH = H // P       # 8 k-tiles over H
F = 4             # D-fold factor: d = f*(D//F) + r; fattens DMA lines 4x
DR = D // F       # 512 folded rows
KF = DR // P      # 4 row-tiles over folded D
R_CHUNKS = [(0, 384), (384, 172)]   # routed-phase column chunks (PSUM bank <= 512 f32;
                                    # big chunk first: c0-g0 consumption time
                                    # sets the opening DMA deadlines)
S_CHUNKS = [(0, 256)]               # shared-phase column chunks
# GEMM1 hidden-dim weight groups. NG=2 matches the opening's issue rate:
# the PE consumes one (x, wg) fold-slice pair per ~1.28us and sync issues
# one pair per ~1.26us. NG=4 (tried: +6.5us) halves PE time per slice and
# starves the opening 2:1.
NG = 2
BWF = 2 * H // NG  # per-fold block width of a w13 group (w1 part + w3 part)

_COMPILED = {}     # build_key -> (nc, tmpdir)
LAST_RESULTS = None  # BassKernelResults of the most recent device run (for test.py)


def _ensure_axon_hooks():
    """This image's antenv lacks axon_hooks, which run_bass_kernel_spmd
    imports unconditionally when tracing. Provide it, wired to the
    libaxon_pjrt ctypes NTFF hook when available."""
    try:
        import antenv.axon_hooks  # noqa: F401
        return
    except ImportError:
        pass
    import types

    try:
        import antenv
    except ImportError:
        return
    mod = types.ModuleType("antenv.axon_hooks")
    holder = {"hook": None}
    mod.set_axon_ntff_profile_hook = lambda h: holder.__setitem__("hook", h)
    mod.get_axon_ntff_profile_hook = lambda: holder["hook"]
    sys.modules["antenv.axon_hooks"] = mod
    antenv.axon_hooks = mod
    try:
        from trn_agent_boot.trn_boot import _ntff_profile_via_ctypes

        hook = _ntff_profile_via_ctypes("/opt/axon/libaxon_pjrt.so")
        if hook is not None:
            mod.set_axon_ntff_profile_hook(hook)
    except Exception:
        pass


_ensure_axon_hooks()


def _build_nc():
    import concourse.bass as bass  # noqa: F401
    import concourse.tile as tile
    from concourse import bacc, mybir

    bf = mybir.dt.bfloat16
    f32 = mybir.dt.float32
    act = mybir.ActivationFunctionType

    nc = bacc.Bacc("TRN2", target_bir_lowering=False, debug=False, num_devices=8)

    # Folded-D DRAM layouts (see kernel() host packing):
    #   x:   [DR, F*n_cols] — per chunk, F column-blocks of that chunk's cols
    #   w13: per group g in {0,1}: rows of [DR, F*H]; within a column block f,
    #        cols [0:GH*P) are w1's group-half, [GH*P:H) are w3's.
    # Folding multiplies DMA line length by F (4), cutting per-packet DMA
    # overhead; the contraction over D becomes a loop over (row-tile, fold).
    xr = nc.dram_tensor("xr", [DR, F * C], bf, kind="ExternalInput").ap()
    xs = nc.dram_tensor("xs", [DR, F * S], bf, kind="ExternalInput").ap()
    w13 = nc.dram_tensor("w13", [NG, DR, F * BWF], bf, kind="ExternalInput").ap()
    w2 = nc.dram_tensor("w2", [H, D], bf, kind="ExternalInput").ap()
    sw13 = nc.dram_tensor("sw13", [NG, DR, F * BWF], bf,
                          kind="ExternalInput").ap()
    sw2 = nc.dram_tensor("sw2", [H, D], bf, kind="ExternalInput").ap()
    # Outputs use the same folded layout as x (unfolded host-side): 4 om-rows
    # share one SBUF staging tile so each store is one DMA with F-times
    # longer lines.
    o_r = nc.dram_tensor("o_r", [DR, F * C], bf, kind="ExternalOutput").ap()
    o_s = nc.dram_tensor("o_s", [DR, F * S], bf, kind="ExternalOutput").ap()


    # w13 host layout (see kernel()): NG hidden-dim groups, each group's
    # per-fold block = [w1 quarter | w3 quarter]. NG=4 halves the opening
    # working set (wg-g0 is 2MB instead of 4MB) and relaxes every later
    # weight-group's DMA deadline; per group-chunk only 2*GH=4 PSUM banks
    # are live, so consecutive group-chunks overlap eviction better too.
    GH = KH // NG  # hm-pairs per group

    with tile.TileContext(nc) as tc:
        with (
            tc.tile_pool(name="xp", bufs=10) as xpool,
            tc.tile_pool(name="wg", bufs=14) as wgpool,
            tc.tile_pool(name="w2p", bufs=10) as w2pool,
            tc.tile_pool(name="hp", bufs=18) as hpool,
            tc.tile_pool(name="op", bufs=3) as opool,
            tc.tile_pool(name="ps", bufs=8, space="PSUM") as pspool,
        ):
            def warmup():
                # ~5us of dummy matmuls while the first weight DMAs are in
                # flight: the HAM clock gate needs ~3.4us of sustained PE
                # activity before releasing the 2.4 GHz clock, so spend the
                # unavoidable initial DMA stall warming it on scratch data.
                zt = hpool.tile([P, 288], bf, tag="h", name="warm_x")
                nc.vector.memset(zt[:], 0.0)
                pw = pspool.tile([P, 288], f32, tag="ps", name="warm_ps")
                # 15 iterations (~3.6us at the gated clock) completes the
                # HAM ramp: 11 leaves the first real matmuls at half clock
                # (measured 320ns vs 162ns spacing at N=384); 17 costs more
                # on average than the straggler gaps it absorbs.
                for it in range(15):
                    nc.tensor.matmul(
                        pw[:], zt[:, :P], zt[:], start=(it == 0), stop=(it == 14)
                    )

            def dma_in(dst, src):
                # All DMA issues on sync's HWDGE chain (~0.63us each).
                # Measured dead ends: issuing from ACT queues ahead of the
                # sigmoids and stalls PSUM eviction (+19us); gpsimd SWDGE
                # shares the same 8 depth-1 lanes, moves ~74GB/s, and
                # steals HBM from the critical opening stream (+30us).
                nc.sync.dma_start(dst, src)

            def ffn(x_dram, n_cols, chunks, w13_dram, w2_dram, out_dram, out_dt,
                    first_phase=False, split_out=False):
                x_sb = {}   # (chunk_idx, kt) -> tile [P, F*nw]
                wg_sb = [[None] * KF for _ in range(NG)]
                w2_sb = [None] * KH

                def load_x(ci):
                    n0, nw = chunks[ci]
                    for kt in range(KF):
                        t = xpool.tile([P, F * nw], bf, tag="x",
                                       name=f"x_{ci}_{kt}")
                        dma_in(t[:], x_dram[kt * P:(kt + 1) * P,
                                            F * n0:F * n0 + F * nw])
                        x_sb[(ci, kt)] = t

                def load_wg(g):
                    for kt in range(KF):
                        w = wgpool.tile([P, F * BWF], bf, tag="wg",
                                        name=f"wg{g}_{kt}")
                        dma_in(w[:], w13_dram[g, kt * P:(kt + 1) * P, :])
                        wg_sb[g][kt] = w

                # Load order tracks PE consumption (group-outer GEMM1, all
                # chunks per group): chunk-0 x + wg-g0 interleaved as the
                # opening, then later chunks' x, then wg-g1..g3 in
                # consumption order, then w2 (needed ~72us). The 8 HWDGE
                # lanes are depth-1 — a lane's next issue waits out its
                # previous transfer — and share HBM ~equally while in
                # flight, so early-needed tiles are split small to keep
                # arrival smooth.
                if first_phase:
                    # opening: per-fold sub-DMAs in PE consumption order.
                    # A full 1.4MB (x,wg) tile pair takes ~8us to land with
                    # 8 transfers sharing HBM, while warmup ends at ~11us —
                    # ~0.36MB slices land from ~9.5us on and keep the PE
                    # fed at one (x,w) slice pair per ~1.26us of issue.
                    n0, nw = chunks[0]
                    for kt in range(KF):
                        xt0 = xpool.tile([P, F * nw], bf, tag="x",
                                         name=f"x_0_{kt}")
                        wt0 = wgpool.tile([P, F * BWF], bf, tag="wg",
                                          name=f"wg0_{kt}")
                        rows = slice(kt * P, (kt + 1) * P)
                        # strict per-fold (x, w) alternation for every kt.
                        # Coarsening kt2/kt3's x to halves (tried) closes
                        # the late kt3 lane waits but re-opens first-slice
                        # and kt1/kt2 gaps: the chain is globally
                        # issue-rate-bound, local compression just moves
                        # the starvation earlier.
                        for f in range(F):
                            dma_in(xt0[:, f * nw:(f + 1) * nw],
                                   x_dram[rows, F * n0 + f * nw:
                                          F * n0 + (f + 1) * nw])
                            dma_in(wt0[:, f * BWF:(f + 1) * BWF],
                                   w13_dram[0, rows, f * BWF:(f + 1) * BWF])
                        x_sb[(0, kt)] = xt0
                        wg_sb[0][kt] = wt0
                else:
                    load_x(0)
                    load_wg(0)
                for ci in range(1, len(chunks)):
                    load_x(ci)
                for g in range(1, NG):
                    load_wg(g)
                for k2 in range(KH):
                    t = w2pool.tile([P, D], bf, tag="w2", name=f"w2_{k2}")
                    dma_in(t[:], w2_dram[k2 * P:(k2 + 1) * P, :])
                    w2_sb[k2] = t

                # All chunks' GEMM1 first, then all chunks' GEMM2: pushes the
                # w2 weight deadline far enough out for DMA to keep ahead of
                # the PE during the DMA-heavy opening phase.
                # group-outer GEMM1: [c0-g0, c1-g0, c0-g1, c1-g1]. Both
                # chunks consume wg0 before wg1 is touched, pushing wg1's
                # DMA deadline from ~33us to ~42us — the opening is
                # aggregate-HBM-bound, so deadline slack is what matters.
                h_by_chunk = [[None] * KH for _ in chunks]
                for g in range(NG):
                    for ci, (n0, nw) in enumerate(chunks):
                        pg1 = [
                            pspool.tile([P, nw], f32, tag="ps",
                                        name=f"pg1_{ci}_{g}_{j}")
                            for j in range(GH)
                        ]
                        pg3 = [
                            pspool.tile([P, nw], f32, tag="ps",
                                        name=f"pg3_{ci}_{g}_{j}")
                            for j in range(GH)
                        ]
                        for kt in range(KF):
                            wt = wg_sb[g][kt]
                            xt_ = x_sb[(ci, kt)]
                            for f in range(F):
                                xsl = xt_[:, f * nw:(f + 1) * nw]
                                first = (kt == 0 and f == 0)
                                last = (kt == KF - 1 and f == F - 1)
                                for j in range(GH):
                                    nc.tensor.matmul(
                                        pg1[j][:],
                                        wt[:, f * BWF + j * P:
                                           f * BWF + (j + 1) * P],
                                        xsl,
                                        start=first, stop=last,
                                    )
                                    nc.tensor.matmul(
                                        pg3[j][:],
                                        wt[:, f * BWF + GH * P + j * P:
                                           f * BWF + GH * P + (j + 1) * P],
                                        xsl,
                                        start=first, stop=last,
                                    )
                        for j in range(GH):
                            # fused silu on ACT directly off PSUM: halves
                            # the eviction latency vs sigmoid+mul+mul, so
                            # PSUM banks recycle faster at group boundaries
                            s_sb = hpool.tile([P, nw], bf, tag="h")
                            nc.scalar.activation(s_sb[:], pg1[j][:], act.Silu)
                            h = hpool.tile([P, nw], bf, tag="h")
                            nc.vector.tensor_mul(h[:], s_sb[:], pg3[j][:])
                            h_by_chunk[ci][g * GH + j] = h
                for (n0, nw), h_sb in zip(chunks, h_by_chunk):
                    for gr in range(KF):
                        o = opool.tile([P, F * nw], out_dt, tag="o",
                                       name=f"o_{n0}_{gr}")
                        for fd in range(F):
                            om = fd * KF + gr  # d rows [om*P, om*P+P)
                            po = pspool.tile([P, nw], f32, tag="ps",
                                             name=f"po_{om}")
                            for kt in range(KH):
                                nc.tensor.matmul(
                                    po[:],
                                    w2_sb[kt][:, om * P:(om + 1) * P],
                                    h_sb[kt][:],
                                    start=(kt == 0), stop=(kt == KH - 1),
                                )
                            nc.vector.tensor_copy(
                                o[:, fd * nw:(fd + 1) * nw], po[:]
                            )
                            if split_out:
                                # last phase: stream each fold block out as
                                # soon as it is evicted — tail latency beats
                                # line efficiency at kernel end
                                dma_in(
                                    out_dram[gr * P:(gr + 1) * P,
                                             F * n0 + fd * nw:
                                             F * n0 + (fd + 1) * nw],
                                    o[:, fd * nw:(fd + 1) * nw],
                                )
                        if not split_out:
                            dma_in(
                                out_dram[gr * P:(gr + 1) * P,
                                         F * n0:F * n0 + F * nw],
                                o[:],
                            )

            warmup()
            ffn(xr, C, R_CHUNKS, w13, w2, o_r, bf, first_phase=True)
            ffn(xs, S, S_CHUNKS, sw13, sw2, o_s, bf, split_out=True)

    nc.compile()
    return nc


def _get_compiled():
    if "nc" not in _COMPILED:
        _COMPILED["nc"] = _build_nc()
        _COMPILED["tmpdir"] = tempfile.mkdtemp(prefix="moe_bass_")
    return _COMPILED["nc"], _COMPILED["tmpdir"]


def _route_host(x, gate, expert_bias):
    """Reference-exact routing on CPU jax: scores, top-2 selection, stable
    sort by expert. Returns (token_idx, expert_ids, scores_sorted) in
    sorted-slot order."""
    import jax
    import jax.numpy as jnp

    cpu = jax.devices("cpu")[0]
    with jax.default_device(cpu):
        xt = jnp.asarray(x.reshape(-1, D))
        scores = jax.nn.sigmoid((xt @ jnp.asarray(gate).T).astype(jnp.float32))
        _, sel = jax.lax.top_k(scores + jnp.asarray(expert_bias)[None, :], TOPK)
        top_scores = jnp.take_along_axis(scores, sel, axis=1) * 1.0
        flat_sel = sel.reshape(-1)
        order = jnp.argsort(flat_sel, stable=True)
        scores_sorted = top_scores.reshape(-1)[order]
        expert_ids = flat_sel[order]
    order = np.asarray(order)
    return (
        order // TOPK,
        np.asarray(expert_ids),
        np.asarray(scores_sorted, dtype=np.float32),
        order,
    )


def _silu32(v):
    return v / (1.0 + np.exp(-v))


def fold_x(x_t, chunks):
    # x_t: [D, n] f32/bf16 -> [DR, F*n] bf16, chunk-major then fold-major
    xf = np.asarray(x_t).reshape(F, DR, x_t.shape[1])
    blocks = [xf[f][:, n0:n0 + nw] for (n0, nw) in chunks for f in range(F)]
    return np.ascontiguousarray(np.concatenate(blocks, axis=1).astype(BF16))


def unfold_x(arr_f, n_cols, chunks):
    # inverse of fold_x: [DR, F*n_cols] -> [D, n_cols]
    out = np.empty((D, n_cols), dtype=arr_f.dtype)
    for (n0, nw) in chunks:
        base = F * n0
        for f in range(F):
            out[f * DR:(f + 1) * DR, n0:n0 + nw] = (
                arr_f[:, base + f * nw:base + (f + 1) * nw]
            )
    return out


def fold_w13(a1, a3):
    # -> [NG, DR, F*BWF]: per hidden-dim group g, fold-major column
    # blocks, each block = [w1 slice | w3 slice] of H//NG rows
    HG = H // NG
    out = np.empty((NG, DR, F * BWF), dtype=BF16)
    for g in range(NG):
        wg = np.concatenate(
            [a1.T[:, g * HG:(g + 1) * HG], a3.T[:, g * HG:(g + 1) * HG]],
            axis=1,
        )  # [D, BWF]
        out[g] = wg.reshape(F, DR, BWF).transpose(1, 0, 2).reshape(DR, F * BWF)
    return out


def _overflow_slots_numpy(xb_rows, w1e, w2e, w3e):
    """Correctness fallback for expert token counts beyond capacity C:
    reproduce the reference's bf16 FFN math in numpy for those rows."""
    a = xb_rows.astype(np.float32)
    g1 = (a @ w1e.astype(BF16).astype(np.float32).T).astype(BF16)
    g3 = (a @ w3e.astype(BF16).astype(np.float32).T).astype(BF16)
    h = (_silu32(g1.astype(np.float32))).astype(BF16).astype(np.float32)
    h = (h * g3.astype(np.float32)).astype(BF16)
    return (h.astype(np.float32) @ w2e.astype(BF16).astype(np.float32).T).astype(
        BF16
    ).astype(np.float32)


def kernel(x, gate, expert_bias, w1, w2, w3, shared_w1, shared_w2, shared_w3):
    global LAST_RESULTS
    from concourse.bass_utils import run_bass_kernel_spmd

    x = np.asarray(x, dtype=np.float32)
    gate = np.asarray(gate, dtype=np.float32)
    expert_bias = np.asarray(expert_bias, dtype=np.float32)
    w1 = np.asarray(w1, dtype=np.float32)
    w2 = np.asarray(w2, dtype=np.float32)
    w3 = np.asarray(w3, dtype=np.float32)
    shared_w1 = np.asarray(shared_w1, dtype=np.float32)
    shared_w2 = np.asarray(shared_w2, dtype=np.float32)
    shared_w3 = np.asarray(shared_w3, dtype=np.float32)

    token_idx, expert_ids, scores_sorted, order = _route_host(x, gate, expert_bias)
    xt = x.reshape(T, D)

    counts = np.bincount(expert_ids, minlength=E)
    offs = np.concatenate([[0], np.cumsum(counts)])

    # Routed tokens, scaled by their gate score then rounded to bf16 exactly
    # like the reference's `routed.astype(bfloat16)`.
    routed_b = (xt[token_idx] * scores_sorted[:, None]).astype(BF16)

    # Shared weights are identical on every core.
    sw13_t = fold_w13(shared_w1, shared_w3)
    sw2_t = np.ascontiguousarray(shared_w2.T.astype(BF16))
    xt_b = xt.astype(BF16)

    in_maps = []
    for e in range(E):
        lo, hi = offs[e], offs[e + 1]
        n_e = min(hi - lo, C)
        xr_t = np.zeros((D, C), dtype=BF16)
        xr_t[:, :n_e] = routed_b[lo:lo + n_e].T
        xr_t = fold_x(xr_t, R_CHUNKS)
        xs_t = fold_x(xt_b[e * S:(e + 1) * S].T, S_CHUNKS)
        w13_t = fold_w13(w1[e], w3[e])
        w2_t = np.ascontiguousarray(w2[e].T.astype(BF16))
        in_maps.append(
            {
                "xr": xr_t,
                "xs": xs_t,
                "w13": w13_t,
                "w2": w2_t,
                "sw13": sw13_t,
                "sw2": sw2_t,
            }
        )

    nc, _ = _get_compiled()
    # fresh tmpdir per call: NTFF profile artifacts collide on reuse
    tmpdir = tempfile.mkdtemp(prefix="moe_bass_")
    res = run_bass_kernel_spmd(nc, in_maps, core_ids=list(range(E)), tmpdir=tmpdir)
    LAST_RESULTS = res

    # Reassemble: shared output slices (f32) + scatter-add of routed outputs.
    out = np.empty((T, D), dtype=np.float32)
    for e in range(E):
        out[e * S:(e + 1) * S] = (
            unfold_x(res.results[e]["o_s"], S, S_CHUNKS).T.astype(np.float32)
        )

    out_r = np.empty((T * TOPK, D), dtype=np.float32)
    for e in range(E):
        lo, hi = offs[e], offs[e + 1]
        n_e = min(hi - lo, C)
        o_r_e = unfold_x(res.results[e]["o_r"], C, R_CHUNKS)
        out_r[lo:lo + n_e] = o_r_e[:, :n_e].T.astype(np.float32)
        if hi - lo > C:  # capacity overflow: exact numpy fallback
            rows = routed_b[lo + C:hi]
            out_r[lo + C:hi] = _overflow_slots_numpy(rows, w1[e], w2[e], w3[e])

    # slot s (sorted order) came from original flat slot order[s]; invert so
    # each token's two expert outputs can be summed with one gather.
    pos = np.empty(T * TOPK, dtype=np.int64)
    pos[order] = np.arange(T * TOPK)
    out += out_r[pos].reshape(T, TOPK, D).sum(axis=1)

    return out.reshape(4, 512, D)



# revision 58
# speedup vs baseline: 1.0536x; 1.0536x over previous
# MoE (top-2 routed experts + shared expert SwiGLU) on 8 TRN2 NeuronCores.
#
# Sharding: expert-parallel. Core e owns expert e's FFN weights and processes
# the tokens routed to expert e (padded to a fixed capacity); the shared
# expert runs data-parallel (each core takes T/8 tokens with replicated
# shared weights). Routing (sigmoid gate -> top-2 -> stable sort by expert)
# is part of the host-side sharding step: it decides which token goes to
# which core, exactly mirroring the reference's jax ops so expert selection
# is bit-identical. All FFN GEMMs (99.9% of FLOPs) run on device in bf16
# with fp32 PSUM accumulation, matching the reference's bf16 expert compute.
#
# Device layout: tokens live on the matmul free dim (everything pre-transposed
# host-side), weights stream as [128, free] k-tiles used as lhsT slices.
import os
import sys
import tempfile

import numpy as np
import ml_dtypes

for _p in ("/opt/trn_rl_repo", "/root/.axon_site/_ro/trn_rl_repo"):
    if os.path.isdir(_p) and _p not in sys.path:
        sys.path.append(_p)

BF16 = ml_dtypes.bfloat16

P = 128
D = 2048          # model dim
H = 1024          # ffn hidden dim
T = 2048          # batch*seq tokens
E = 8             # experts == cores
TOPK = 2
C = 556           # per-expert token capacity (max observed count 554; overflow has a numpy fallback)
S = T // 8        # shared-expert tokens per core
KD = D // P       # 16 k-tiles over D
KH = H // P       # 8 k-tiles over H
F = 4             # D-fold factor: d = f*(D//F) + r; fattens DMA lines 4x
DR = D // F       # 512 folded rows
KF = DR // P      # 4 row-tiles over folded D
R_CHUNKS = [(0, 384), (384, 172)]   # routed-phase column chunks (PSUM bank <= 512 f32;
                                    # big chunk first: c0-g0 consumption time
                                    # sets the opening DMA deadlines)
S_CHUNKS = [(0, 256)]               # shared-phase column chunks
# GEMM1 hidden-dim weight groups. NG=2 matches the opening's issue rate:
# the PE consumes one (x, wg) fold-slice pair per ~1.28us and sync issues
# one pair per ~1.26us. NG=4 (tried: +6.5us) halves PE time per slice and
# starves the opening 2:1.
NG = 2
BWF = 2 * H // NG  # per-fold block width of a w13 group (w1 part + w3 part)

_COMPILED = {}     # build_key -> (nc, tmpdir)
LAST_RESULTS = None  # BassKernelResults of the most recent device run (for test.py)


def _ensure_axon_hooks():
    """This image's antenv lacks axon_hooks, which run_bass_kernel_spmd
    imports unconditionally when tracing. Provide it, wired to the
    libaxon_pjrt ctypes NTFF hook when available."""
    try:
        import antenv.axon_hooks  # noqa: F401
        return
    except ImportError:
        pass
    import types

    try:
        import antenv
    except ImportError:
        return
    mod = types.ModuleType("antenv.axon_hooks")
    holder = {"hook": None}
    mod.set_axon_ntff_profile_hook = lambda h: holder.__setitem__("hook", h)
    mod.get_axon_ntff_profile_hook = lambda: holder["hook"]
    sys.modules["antenv.axon_hooks"] = mod
    antenv.axon_hooks = mod
    try:
        from trn_agent_boot.trn_boot import _ntff_profile_via_ctypes

        hook = _ntff_profile_via_ctypes("/opt/axon/libaxon_pjrt.so")
        if hook is not None:
            mod.set_axon_ntff_profile_hook(hook)
    except Exception:
        pass


_ensure_axon_hooks()


def _build_nc():
    import concourse.bass as bass  # noqa: F401
    import concourse.tile as tile
    from concourse import bacc, mybir

    bf = mybir.dt.bfloat16
    f32 = mybir.dt.float32
    act = mybir.ActivationFunctionType

    nc = bacc.Bacc("TRN2", target_bir_lowering=False, debug=False, num_devices=8)

    # Folded-D DRAM layouts (see kernel() host packing):
    #   x:   [DR, F*n_cols] — per chunk, F column-blocks of that chunk's cols
    #   w13: per group g in {0,1}: rows of [DR, F*H]; within a column block f,
    #        cols [0:GH*P) are w1's group-half, [GH*P:H) are w3's.
    # Folding multiplies DMA line length by F (4), cutting per-packet DMA
    # overhead; the contraction over D becomes a loop over (row-tile, fold).
    xr = nc.dram_tensor("xr", [DR, F * C], bf, kind="ExternalInput").ap()
    xs = nc.dram_tensor("xs", [DR, F * S], bf, kind="ExternalInput").ap()
    w13 = nc.dram_tensor("w13", [NG, DR, F * BWF], bf, kind="ExternalInput").ap()
    w2 = nc.dram_tensor("w2", [H, D], bf, kind="ExternalInput").ap()
    sw13 = nc.dram_tensor("sw13", [NG, DR, F * BWF], bf,
                          kind="ExternalInput").ap()
    sw2 = nc.dram_tensor("sw2", [H, D], bf, kind="ExternalInput").ap()
    # Outputs use the same folded layout as x (unfolded host-side): 4 om-rows
    # share one SBUF staging tile so each store is one DMA with F-times
    # longer lines.
    o_r = nc.dram_tensor("o_r", [DR, F * C], bf, kind="ExternalOutput").ap()
    o_s = nc.dram_tensor("o_s", [DR, F * S], bf, kind="ExternalOutput").ap()


    # w13 host layout (see kernel()): NG hidden-dim groups, each group's
    # per-fold block = [w1 quarter | w3 quarter]. NG=4 halves the opening
    # working set (wg-g0 is 2MB instead of 4MB) and relaxes every later
    # weight-group's DMA deadline; per group-chunk only 2*GH=4 PSUM banks
    # are live, so consecutive group-chunks overlap eviction better too.
    GH = KH // NG  # hm-pairs per group

    with tile.TileContext(nc) as tc:
        with (
            tc.tile_pool(name="xp", bufs=10) as xpool,
            tc.tile_pool(name="wg", bufs=14) as wgpool,
            tc.tile_pool(name="w2p", bufs=10) as w2pool,
            tc.tile_pool(name="hp", bufs=18) as hpool,
            tc.tile_pool(name="op", bufs=3) as opool,
            tc.tile_pool(name="ps", bufs=8, space="PSUM") as pspool,
        ):
            def warmup():
                # ~5us of dummy matmuls while the first weight DMAs are in
                # flight: the HAM clock gate needs ~3.4us of sustained PE
                # activity before releasing the 2.4 GHz clock, so spend the
                # unavoidable initial DMA stall warming it on scratch data.
                zt = hpool.tile([P, 288], bf, tag="h", name="warm_x")
                nc.vector.memset(zt[:], 0.0)
                pw = pspool.tile([P, 288], f32, tag="ps", name="warm_ps")
                # 15 iterations (~3.6us at the gated clock) completes the
                # HAM ramp: 11 leaves the first real matmuls at half clock
                # (measured 320ns vs 162ns spacing at N=384); 17 costs more
                # on average than the straggler gaps it absorbs.
                for it in range(15):
                    nc.tensor.matmul(
                        pw[:], zt[:, :P], zt[:], start=(it == 0), stop=(it == 14)
                    )

            def dma_in(dst, src):
                # All DMA issues on sync's HWDGE chain (~0.63us each).
                # Measured dead ends: issuing from ACT queues ahead of the
                # sigmoids and stalls PSUM eviction (+19us); gpsimd SWDGE
                # shares the same 8 depth-1 lanes, moves ~74GB/s, and
                # steals HBM from the critical opening stream (+30us).
                nc.sync.dma_start(dst, src)

            def ffn_loads(x_dram, chunks, w13_dram, w2_dram,
                          first_phase=False):
                # Emits only the DMA loads and returns the tile handles.
                # Splitting loads from compute lets the shared phase's
                # loads be emitted before the routed stores: otherwise
                # they sit behind the first o_r store's data-wait (~76us)
                # on sync's in-order chain, leaving HBM idle at 50-60us
                # and bursting afterwards.
                x_sb = {}   # (chunk_idx, kt) -> tile [P, F*nw]
                wg_sb = [[None] * KF for _ in range(NG)]
                w2_sb = [None] * KH

                def load_x(ci):
                    n0, nw = chunks[ci]
                    for kt in range(KF):
                        t = xpool.tile([P, F * nw], bf, tag="x",
                                       name=f"x_{ci}_{kt}")
                        dma_in(t[:], x_dram[kt * P:(kt + 1) * P,
                                            F * n0:F * n0 + F * nw])
                        x_sb[(ci, kt)] = t

                def load_wg(g):
                    for kt in range(KF):
                        w = wgpool.tile([P, F * BWF], bf, tag="wg",
                                        name=f"wg{g}_{kt}")
                        dma_in(w[:], w13_dram[g, kt * P:(kt + 1) * P, :])
                        wg_sb[g][kt] = w

                # Load order tracks PE consumption (group-outer GEMM1, all
                # chunks per group): chunk-0 x + wg-g0 interleaved as the
                # opening, then later chunks' x, then wg-g1..g3 in
                # consumption order, then w2 (needed ~72us). The 8 HWDGE
                # lanes are depth-1 — a lane's next issue waits out its
                # previous transfer — and share HBM ~equally while in
                # flight, so early-needed tiles are split small to keep
                # arrival smooth.
                if first_phase:
                    # opening: per-fold sub-DMAs in PE consumption order.
                    # A full 1.4MB (x,wg) tile pair takes ~8us to land with
                    # 8 transfers sharing HBM, while warmup ends at ~11us —
                    # ~0.36MB slices land from ~9.5us on and keep the PE
                    # fed at one (x,w) slice pair per ~1.26us of issue.
                    n0, nw = chunks[0]
                    for kt in range(KF):
                        xt0 = xpool.tile([P, F * nw], bf, tag="x",
                                         name=f"x_0_{kt}")
                        wt0 = wgpool.tile([P, F * BWF], bf, tag="wg",
                                          name=f"wg0_{kt}")
                        rows = slice(kt * P, (kt + 1) * P)
                        # strict per-fold (x, w) alternation for every kt.
                        # Coarsening kt2/kt3's x to halves (tried) closes
                        # the late kt3 lane waits but re-opens first-slice
                        # and kt1/kt2 gaps: the chain is globally
                        # issue-rate-bound, local compression just moves
                        # the starvation earlier.
                        for f in range(F):
                            dma_in(xt0[:, f * nw:(f + 1) * nw],
                                   x_dram[rows, F * n0 + f * nw:
                                          F * n0 + (f + 1) * nw])
                            dma_in(wt0[:, f * BWF:(f + 1) * BWF],
                                   w13_dram[0, rows, f * BWF:(f + 1) * BWF])
                        x_sb[(0, kt)] = xt0
                        wg_sb[0][kt] = wt0
                else:
                    load_x(0)
                    load_wg(0)
                for ci in range(1, len(chunks)):
                    load_x(ci)
                for g in range(1, NG):
                    load_wg(g)
                for k2 in range(KH):
                    t = w2pool.tile([P, D], bf, tag="w2", name=f"w2_{k2}")
                    dma_in(t[:], w2_dram[k2 * P:(k2 + 1) * P, :])
                    w2_sb[k2] = t
                return x_sb, wg_sb, w2_sb

            def ffn_compute(tiles, chunks, out_dram, out_dt,
                            split_out=False):
                x_sb, wg_sb, w2_sb = tiles
                # All chunks' GEMM1 first, then all chunks' GEMM2: pushes the
                # w2 weight deadline far enough out for DMA to keep ahead of
                # the PE during the DMA-heavy opening phase.
                # group-outer GEMM1: [c0-g0, c1-g0, c0-g1, c1-g1]. Both
                # chunks consume wg0 before wg1 is touched, pushing wg1's
                # DMA deadline from ~33us to ~42us — the opening is
                # aggregate-HBM-bound, so deadline slack is what matters.
                h_by_chunk = [[None] * KH for _ in chunks]
                for g in range(NG):
                    for ci, (n0, nw) in enumerate(chunks):
                        pg1 = [
                            pspool.tile([P, nw], f32, tag="ps",
                                        name=f"pg1_{ci}_{g}_{j}")
                            for j in range(GH)
                        ]
                        pg3 = [
                            pspool.tile([P, nw], f32, tag="ps",
                                        name=f"pg3_{ci}_{g}_{j}")
                            for j in range(GH)
                        ]
                        for kt in range(KF):
                            wt = wg_sb[g][kt]
                            xt_ = x_sb[(ci, kt)]
                            for f in range(F):
                                xsl = xt_[:, f * nw:(f + 1) * nw]
                                first = (kt == 0 and f == 0)
                                last = (kt == KF - 1 and f == F - 1)
                                for j in range(GH):
                                    nc.tensor.matmul(
                                        pg1[j][:],
                                        wt[:, f * BWF + j * P:
                                           f * BWF + (j + 1) * P],
                                        xsl,
                                        start=first, stop=last,
                                    )
                                    nc.tensor.matmul(
                                        pg3[j][:],
                                        wt[:, f * BWF + GH * P + j * P:
                                           f * BWF + GH * P + (j + 1) * P],
                                        xsl,
                                        start=first, stop=last,
                                    )
                        for j in range(GH):
                            # fused silu on ACT directly off PSUM: halves
                            # the eviction latency vs sigmoid+mul+mul, so
                            # PSUM banks recycle faster at group boundaries
                            s_sb = hpool.tile([P, nw], bf, tag="h")
                            nc.scalar.activation(s_sb[:], pg1[j][:], act.Silu)
                            h = hpool.tile([P, nw], bf, tag="h")
                            nc.vector.tensor_mul(h[:], s_sb[:], pg3[j][:])
                            h_by_chunk[ci][g * GH + j] = h
                for (n0, nw), h_sb in zip(chunks, h_by_chunk):
                    for gr in range(KF):
                        o = opool.tile([P, F * nw], out_dt, tag="o",
                                       name=f"o_{n0}_{gr}")
                        for fd in range(F):
                            om = fd * KF + gr  # d rows [om*P, om*P+P)
                            po = pspool.tile([P, nw], f32, tag="ps",
                                             name=f"po_{om}")
                            for kt in range(KH):
                                nc.tensor.matmul(
                                    po[:],
                                    w2_sb[kt][:, om * P:(om + 1) * P],
                                    h_sb[kt][:],
                                    start=(kt == 0), stop=(kt == KH - 1),
                                )
                            nc.vector.tensor_copy(
                                o[:, fd * nw:(fd + 1) * nw], po[:]
                            )
                            if split_out:
                                # last phase: stream each fold block out as
                                # soon as it is evicted — tail latency beats
                                # line efficiency at kernel end
                                dma_in(
                                    out_dram[gr * P:(gr + 1) * P,
                                             F * n0 + fd * nw:
                                             F * n0 + (fd + 1) * nw],
                                    o[:, fd * nw:(fd + 1) * nw],
                                )
                        if not split_out:
                            dma_in(
                                out_dram[gr * P:(gr + 1) * P,
                                         F * n0:F * n0 + F * nw],
                                o[:],
                            )

            warmup()
            r_tiles = ffn_loads(xr, R_CHUNKS, w13, w2, first_phase=True)
            # shared loads emitted before the routed stores: they have no
            # data deps, so they issue into the 40-70us HBM lull instead
            # of queueing behind the first o_r store's ~76us data-wait.
            # (Pool rings are full, so some reuse-waits apply — still far
            # earlier than today's post-store issue point.)
            s_tiles = ffn_loads(xs, S_CHUNKS, sw13, sw2)
            ffn_compute(r_tiles, R_CHUNKS, o_r, bf)
            ffn_compute(s_tiles, S_CHUNKS, o_s, bf, split_out=True)

    nc.compile()
    return nc


def _get_compiled():
    if "nc" not in _COMPILED:
        _COMPILED["nc"] = _build_nc()
        _COMPILED["tmpdir"] = tempfile.mkdtemp(prefix="moe_bass_")
    return _COMPILED["nc"], _COMPILED["tmpdir"]


def _route_host(x, gate, expert_bias):
    """Reference-exact routing on CPU jax: scores, top-2 selection, stable
    sort by expert. Returns (token_idx, expert_ids, scores_sorted) in
    sorted-slot order."""
    import jax
    import jax.numpy as jnp

    cpu = jax.devices("cpu")[0]
    with jax.default_device(cpu):
        xt = jnp.asarray(x.reshape(-1, D))
        scores = jax.nn.sigmoid((xt @ jnp.asarray(gate).T).astype(jnp.float32))
        _, sel = jax.lax.top_k(scores + jnp.asarray(expert_bias)[None, :], TOPK)
        top_scores = jnp.take_along_axis(scores, sel, axis=1) * 1.0
        flat_sel = sel.reshape(-1)
        order = jnp.argsort(flat_sel, stable=True)
        scores_sorted = top_scores.reshape(-1)[order]
        expert_ids = flat_sel[order]
    order = np.asarray(order)
    return (
        order // TOPK,
        np.asarray(expert_ids),
        np.asarray(scores_sorted, dtype=np.float32),
        order,
    )


def _silu32(v):
    return v / (1.0 + np.exp(-v))


def fold_x(x_t, chunks):
    # x_t: [D, n] f32/bf16 -> [DR, F*n] bf16, chunk-major then fold-major
    xf = np.asarray(x_t).reshape(F, DR, x_t.shape[1])
    blocks = [xf[f][:, n0:n0 + nw] for (n0, nw) in chunks for f in range(F)]
    return np.ascontiguousarray(np.concatenate(blocks, axis=1).astype(BF16))


def unfold_x(arr_f, n_cols, chunks):
    # inverse of fold_x: [DR, F*n_cols] -> [D, n_cols]
    out = np.empty((D, n_cols), dtype=arr_f.dtype)
    for (n0, nw) in chunks:
        base = F * n0
        for f in range(F):
            out[f * DR:(f + 1) * DR, n0:n0 + nw] = (
                arr_f[:, base + f * nw:base + (f + 1) * nw]
            )
    return out


def fold_w13(a1, a3):
    # -> [NG, DR, F*BWF]: per hidden-dim group g, fold-major column
    # blocks, each block = [w1 slice | w3 slice] of H//NG rows
    HG = H // NG
    out = np.empty((NG, DR, F * BWF), dtype=BF16)
    for g in range(NG):
        wg = np.concatenate(
            [a1.T[:, g * HG:(g + 1) * HG], a3.T[:, g * HG:(g + 1) * HG]],
            axis=1,
        )  # [D, BWF]
        out[g] = wg.reshape(F, DR, BWF).transpose(1, 0, 2).reshape(DR, F * BWF)
    return out


def _overflow_slots_numpy(xb_rows, w1e, w2e, w3e):
    """Correctness fallback for expert token counts beyond capacity C:
    reproduce the reference's bf16 FFN math in numpy for those rows."""
    a = xb_rows.astype(np.float32)
    g1 = (a @ w1e.astype(BF16).astype(np.float32).T).astype(BF16)
    g3 = (a @ w3e.astype(BF16).astype(np.float32).T).astype(BF16)
    h = (_silu32(g1.astype(np.float32))).astype(BF16).astype(np.float32)
    h = (h * g3.astype(np.float32)).astype(BF16)
    return (h.astype(np.float32) @ w2e.astype(BF16).astype(np.float32).T).astype(
        BF16
    ).astype(np.float32)


def kernel(x, gate, expert_bias, w1, w2, w3, shared_w1, shared_w2, shared_w3):
    global LAST_RESULTS
    from concourse.bass_utils import run_bass_kernel_spmd

    x = np.asarray(x, dtype=np.float32)
    gate = np.asarray(gate, dtype=np.float32)
    expert_bias = np.asarray(expert_bias, dtype=np.float32)
    w1 = np.asarray(w1, dtype=np.float32)
    w2 = np.asarray(w2, dtype=np.float32)
    w3 = np.asarray(w3, dtype=np.float32)
    shared_w1 = np.asarray(shared_w1, dtype=np.float32)
    shared_w2 = np.asarray(shared_w2, dtype=np.float32)
    shared_w3 = np.asarray(shared_w3, dtype=np.float32)

    token_idx, expert_ids, scores_sorted, order = _route_host(x, gate, expert_bias)
    xt = x.reshape(T, D)

    counts = np.bincount(expert_ids, minlength=E)
    offs = np.concatenate([[0], np.cumsum(counts)])

    # Routed tokens, scaled by their gate score then rounded to bf16 exactly
    # like the reference's `routed.astype(bfloat16)`.
    routed_b = (xt[token_idx] * scores_sorted[:, None]).astype(BF16)

    # Shared weights are identical on every core.
    sw13_t = fold_w13(shared_w1, shared_w3)
    sw2_t = np.ascontiguousarray(shared_w2.T.astype(BF16))
    xt_b = xt.astype(BF16)

    in_maps = []
    for e in range(E):
        lo, hi = offs[e], offs[e + 1]
        n_e = min(hi - lo, C)
        xr_t = np.zeros((D, C), dtype=BF16)
        xr_t[:, :n_e] = routed_b[lo:lo + n_e].T
        xr_t = fold_x(xr_t, R_CHUNKS)
        xs_t = fold_x(xt_b[e * S:(e + 1) * S].T, S_CHUNKS)
        w13_t = fold_w13(w1[e], w3[e])
        w2_t = np.ascontiguousarray(w2[e].T.astype(BF16))
        in_maps.append(
            {
                "xr": xr_t,
                "xs": xs_t,
                "w13": w13_t,
                "w2": w2_t,
                "sw13": sw13_t,
                "sw2": sw2_t,
            }
        )

    nc, _ = _get_compiled()
    # fresh tmpdir per call: NTFF profile artifacts collide on reuse
    tmpdir = tempfile.mkdtemp(prefix="moe_bass_")
    res = run_bass_kernel_spmd(nc, in_maps, core_ids=list(range(E)), tmpdir=tmpdir)
    LAST_RESULTS = res

    # Reassemble: shared output slices (f32) + scatter-add of routed outputs.
    out = np.empty((T, D), dtype=np.float32)
    for e in range(E):
        out[e * S:(e + 1) * S] = (
            unfold_x(res.results[e]["o_s"], S, S_CHUNKS).T.astype(np.float32)
        )

    out_r = np.empty((T * TOPK, D), dtype=np.float32)
    for e in range(E):
        lo, hi = offs[e], offs[e + 1]
        n_e = min(hi - lo, C)
        o_r_e = unfold_x(res.results[e]["o_r"], C, R_CHUNKS)
        out_r[lo:lo + n_e] = o_r_e[:, :n_e].T.astype(np.float32)
        if hi - lo > C:  # capacity overflow: exact numpy fallback
            rows = routed_b[lo + C:hi]
            out_r[lo + C:hi] = _overflow_slots_numpy(rows, w1[e], w2[e], w3[e])

    # slot s (sorted order) came from original flat slot order[s]; invert so
    # each token's two expert outputs can be summed with one gather.
    pos = np.empty(T * TOPK, dtype=np.int64)
    pos[order] = np.arange(T * TOPK)
    out += out_r[pos].reshape(T, TOPK, D).sum(axis=1)

    return out.reshape(4, 512, D)

